# revision 27
# baseline (speedup 1.0000x reference)
"""Chamfer/KNN top-4 mean distance kernel for Trainium2 (8 NeuronCores).

Problem: query [4, 8192, 3], ref [4, 8192, 3], K=4.
  d2[b,n,m] = ||q_bn - r_bm||^2 ; answer = mean over (b,n) of the 4 smallest
  d2[b,n,:] values.

Strategy:
  - Augmented-matmul distances: q' = [2q, -||q||^2, -1], r' = [r, 1, ||r||^2]
    so the PE matmul q'^T @ r' yields NEGATED squared distances in PSUM, and
    the DVE `max` (hardware top-8, descending) extracts the 4 smallest d2
    per query in one pass.
  - 2D locality sharding (host-side layout): queries of each batch are
    sorted into 8 x-strips, y-sorted within each strip, and cut into
    128-query tiles. Each tile is paired with the W=2048 refs closest to
    its bounding box (smallest box-expansion radius). A per-query guard —
    dist(q, box boundary)^2 >= found 4th-smallest d2 — proves exactness;
    the ~0.3% of queries failing the guard are recomputed exactly on the
    host against the full ref set.
  - 8 cores: 2 per batch, 32 tiles each. Per tile: one region DMA
    ([5, 128+W]), a 1x1 dummy matmul that absorbs the DMA semaphore wait
    (walrus allows a single sync wait on fp32 Matmult), 4 matmuls of
    [128, 512] into two 2-bank PSUM buffers, and two `max` ops writing
    top-8 candidates straight into the output tile.
  - Host merges each tile's two top-8 halves, applies the guard, patches
    failures, and averages.
"""

import numpy as np

import concourse.bass as bass
import concourse.mybir as mybir
import concourse.tile as tile
from concourse.bass_utils import run_bass_kernel_spmd

N_CORES = 8
B, N, M, D = 4, 8192, 8192, 3
NQ = 4096       # query rows per core
QT = 128        # queries per tile (PSUM partition dim)
NT = NQ // QT   # 32 tiles per core
W = 2048        # refs per tile window
HALF = W // 2   # refs per PSUM buffer (2 banks)
CHUNK = 512     # matmul free dim (one PSUM bank, fp32)
RS = QT + W     # region stride: [queries | window refs]
NSTRIP = 8      # x-strips per batch
GUARD_EPS = 1e-3


def _build_nc(loop_n=None):
    f32 = mybir.dt.float32
    f32r = mybir.dt.float32r
    nc = bass.Bass()
    qr_d = nc.dram_tensor("qr", [5, NT * RS], f32r, kind="ExternalInput")
    o_d = nc.dram_tensor("o", [QT, NT * 16], f32, kind="ExternalOutput")

    with tile.TileContext(nc) as tc:
        with (
            tc.tile_pool(name="reg", bufs=4) as rpool,
            tc.tile_pool(name="acc", bufs=1) as apool,
            tc.tile_pool(name="psum", bufs=3, space="PSUM") as ppool,
            tc.tile_pool(name="scratch", bufs=1, space="PSUM") as spool,
        ):
            def body():
                vals = apool.tile([QT, NT * 16], f32, tag="vals")
                scratch = spool.tile([QT, 8], f32, tag="scratch")
                for t in range(NT):
                    rg = rpool.tile([5, RS], f32r, tag="rg")
                    # HWDGE (sync engine) region load; a [5, RS] transfer
                    # lands on a single HW queue/semaphore, queues rotate
                    # across tiles so transfers overlap
                    nc.sync.dma_start(rg[:], qr_d[:, t * RS:(t + 1) * RS])
                    # 1x1 dummy matmul: absorbs the DMA-semaphore wait on
                    # PE so the real matmuls below carry at most one wait
                    # (the PSUM-slot recycle wait) — walrus limit.
                    nc.tensor.matmul(
                        scratch[0:1, 0:1],
                        rg[0:1, 0:1].bitcast(f32),
                        rg[0:1, 0:1].bitcast(f32),
                    )
                    # float32r runs the PE at 1 cycle/row (fp32 pays 4x);
                    # the ~1e-4 abs distance error is far below GUARD_EPS
                    # and irrelevant to the final mean.
                    w_ap = rg[:, 0:QT]
                    for h in range(2):
                        ps = ppool.tile([QT, HALF], f32, tag="ps")
                        for c in range(HALF // CHUNK):
                            lo = QT + h * HALF + c * CHUNK
                            nc.tensor.matmul(
                                ps[:, c * CHUNK:(c + 1) * CHUNK],
                                w_ap,
                                rg[:, lo:lo + CHUNK],
                            )
                        # top-8 of -d2 (descending) -> 8 smallest d2 of
                        # this half-window, straight into the output tile
                        nc.vector.max(
                            vals[:, t * 16 + h * 8:t * 16 + (h + 1) * 8],
                            ps[:],
                        )
                nc.sync.dma_start(o_d[:], vals[:])

            for _rep in range(loop_n or 1):  # loop_n: timing harness only
                body()

    # Walrus allows only ONE sync wait on a (self-loading) fp32 Matmult and
    # few on a Drain; Tile's wait pruning is disabled upstream, so prune:
    #  - Matmult: drop same-engine PE waits (PE executes matmuls in order).
    #  - Tail SP Drain: keep only the output-DMA (DMAHW) wait; the rest are
    #    transitively implied by the DMA's own waits.
    # sem updated by the final (output) DMA — the only wait the tail drain
    # needs: output-DMA-complete transitively implies DVE done, PE done,
    # and (via the dummy matmuls) every region DMA complete.
    last_dma_sem = None
    for blk in nc.m.functions[0].blocks:
        for inst in blk.instructions:
            if inst.opcode == "DMACopy" and inst.sync_info is not None:
                for u in inst.sync_info.on_update:
                    last_dma_sem = u.ant_name
    for blk in nc.m.functions[0].blocks:
        for inst in blk.instructions:
            si = inst.sync_info
            if si is None or len(si.on_wait) <= 1:
                continue
            if inst.opcode == "Matmult":
                kept = [w for w in si.on_wait if not w.ant_name.startswith("PE")]
                assert len(kept) <= 1, (
                    f"{inst.name}: {len(kept)} non-PE waits remain"
                )
                si.on_wait = kept
            elif inst.opcode == "DMACopy":
                # region-slot WAW: the PE wait (slot readers done, incl. the
                # dummy matmul that waited on the slot's previous DMA)
                # transitively implies the previous-DMA wait.
                if any(w.ant_name.startswith("PE") for w in si.on_wait):
                    kept = [
                        w for w in si.on_wait
                        if not w.ant_name.startswith(("DMASW", "DMAHW"))
                    ]
                    assert len(kept) <= 1, (
                        f"{inst.name}: {len(kept)} waits remain"
                    )
                    si.on_wait = kept
            elif inst.opcode == "Drain":
                kept = [w for w in si.on_wait if w.ant_name == last_dma_sem]
                if kept and len(kept) < len(si.on_wait):
                    si.on_wait = kept
    return nc


def _aug_q(qs):
    """[n, 3] queries -> [5, n] augmented lhsT columns."""
    out = np.empty((5, qs.shape[0]), dtype=np.float32)
    out[0:3] = 2.0 * qs.T
    out[3] = -np.sum(qs * qs, axis=-1)
    out[4] = -1.0
    return out


def _aug_r(rs):
    """[m, 3] refs -> [5, m] augmented rhs columns."""
    out = np.empty((5, rs.shape[0]), dtype=np.float32)
    out[0:3] = rs.T
    out[3] = 1.0
    out[4] = np.sum(rs * rs, axis=-1)
    return out


def _pack_inputs(query, ref):
    """Build per-core inputs + metadata for the guard/patch step.

    Returns (in_maps, meta) where meta[core] is a list of per-tile dicts:
    {qt: [128,3] query coords, b: batch, box: (xlo, xhi, ylo, yhi)}.
    """
    query = np.ascontiguousarray(np.asarray(query, dtype=np.float32))
    ref = np.ascontiguousarray(np.asarray(ref, dtype=np.float32))
    SPQ = N // NSTRIP  # queries per strip
    TPS = SPQ // QT    # tiles per strip
    in_maps = [
        {"qr": np.empty((5, NT * RS), dtype=np.float32)} for _ in range(N_CORES)
    ]
    meta = [[None] * NT for _ in range(N_CORES)]
    for b in range(B):
        q = query[b]
        r = ref[b]
        qo = np.argsort(q[:, 0], kind="stable")
        qs0 = q[qo]
        tile_idx = 0  # 0..63 within batch
        for st in range(NSTRIP):
            qstrip = qs0[st * SPQ:(st + 1) * SPQ]
            yo = np.argsort(qstrip[:, 1], kind="stable")
            qstrip = qstrip[yo]
            for tt in range(TPS):
                qt = qstrip[tt * QT:(tt + 1) * QT]
                xlo, xhi = float(qt[:, 0].min()), float(qt[:, 0].max())
                ylo, yhi = float(qt[:, 1].min()), float(qt[:, 1].max())
                # box-expansion radius needed to include each ref
                mx = np.maximum(np.maximum(xlo - r[:, 0], r[:, 0] - xhi), 0.0)
                my = np.maximum(np.maximum(ylo - r[:, 1], r[:, 1] - yhi), 0.0)
                mreq = np.maximum(mx, my)
                take = np.argpartition(mreq, W - 1)[:W]
                m_eff = float(mreq[take].max())
                # guard box must be fully covered by the taken refs; ties
                # at m_eff may be split by argpartition, so shrink a hair
                m_guard = max(m_eff * (1.0 - 1e-6) - 1e-9, 0.0)
                rslab = r[take]
                core = 2 * b + (0 if tile_idx < NT else 1)
                t = tile_idx % NT
                reg = in_maps[core]["qr"][:, t * RS:(t + 1) * RS]
                reg[:, 0:QT] = _aug_q(qt)
                reg[:, QT:QT + W] = _aug_r(rslab)
                meta[core][t] = {
                    "qt": qt,
                    "b": b,
                    "box": (xlo - m_guard, xhi + m_guard,
                            ylo - m_guard, yhi + m_guard),
                }
                tile_idx += 1
    return in_maps, meta


def _finish(results, meta, query, ref, K):
    """Merge device top-8 halves, apply exactness guard, patch failures."""
    ref = np.asarray(ref, dtype=np.float32)
    total = 0.0
    count = 0
    n_patched = 0
    for core in range(N_CORES):
        o = results[core]["o"].astype(np.float64)  # [128, NT*16], -d2 desc
        for t in range(NT):
            md = meta[core][t]
            cand = -o[:, t * 16:(t + 1) * 16]  # [128, 16] d2, two sorted 8s
            cand.sort(axis=1)
            top4 = cand[:, :4]
            v4 = top4[:, 3]
            qt = md["qt"].astype(np.float64)
            xlo, xhi, ylo, yhi = md["box"]
            gap = np.minimum.reduce([
                qt[:, 0] - xlo, xhi - qt[:, 0],
                qt[:, 1] - ylo, yhi - qt[:, 1],
            ])
            ok = gap * gap >= v4 + GUARD_EPS
            bad = np.where(~ok)[0]
            if len(bad):
                r = ref[md["b"]].astype(np.float64)
                for p in bad:
                    qrow = qt[p]
                    d2 = np.sum((r - qrow) ** 2, axis=1)
                    top4[p] = np.sort(np.partition(d2, 3)[:4])
                n_patched += len(bad)
            total += float(top4.sum())
            count += QT * 4
    assert count == B * N * int(K)
    _finish.n_patched = n_patched
    return total / count


def kernel(query, ref, K):
    assert int(K) == 4, f"kernel hardcodes K=4, got {K}"
    qa = np.asarray(query)
    assert qa.shape == (B, N, D)
    in_maps, meta = _pack_inputs(query, ref)
    nc = _build_nc()
    res = run_bass_kernel_spmd(nc, in_maps, core_ids=list(range(N_CORES)))
    kernel._last = res  # for test harness introspection
    mean = _finish(res.results, meta, query, ref, K)
    return np.float32(mean)


# revision 29
# speedup vs baseline: 2.0913x; 2.0913x over previous
"""Chamfer/KNN top-4 mean distance kernel for Trainium2 (8 NeuronCores).

Problem: query [4, 8192, 3], ref [4, 8192, 3], K=4.
  d2[b,n,m] = ||q_bn - r_bm||^2 ; answer = mean over (b,n) of the 4 smallest
  d2[b,n,:] values.

Strategy:
  - Augmented-matmul distances: q' = [2q, -||q||^2, -1], r' = [r, 1, ||r||^2]
    so the PE matmul q'^T @ r' yields NEGATED squared distances in PSUM, and
    the DVE `max` (hardware top-8, descending) extracts the 4 smallest d2
    per query in one pass.
  - 2D locality sharding (host-side layout): queries of each batch are
    sorted into 8 x-strips, y-sorted within each strip, and cut into
    128-query tiles. Each tile is paired with the W=2048 refs closest to
    its bounding box (smallest box-expansion radius). A per-query guard —
    dist(q, box boundary)^2 >= found 4th-smallest d2 — proves exactness;
    the ~0.3% of queries failing the guard are recomputed exactly on the
    host against the full ref set.
  - 8 cores: 2 per batch, 32 tiles each. Per tile: one region DMA
    ([5, 128+W]), a 1x1 dummy matmul that absorbs the DMA semaphore wait
    (walrus allows a single sync wait on fp32 Matmult), 4 matmuls of
    [128, 512] into two 2-bank PSUM buffers, and two `max` ops writing
    top-8 candidates straight into the output tile.
  - Host merges each tile's two top-8 halves, applies the guard, patches
    failures, and averages.
"""

import numpy as np

import concourse.bass as bass
import concourse.mybir as mybir
import concourse.tile as tile
from concourse.bass_utils import run_bass_kernel_spmd

N_CORES = 8
B, N, M, D = 4, 8192, 8192, 3
NQ = 4096       # query rows per core
QT = 128        # queries per tile (PSUM partition dim)
NT = NQ // QT   # 32 tiles per core
W = 1280        # refs per tile window
CHUNK = 512     # matmul free dim (one PSUM bank, fp32)
RS = QT + W     # region stride: [queries | window refs]
NSTRIP = 8      # x-strips per batch
GUARD_EPS = 1e-3


def _build_nc(loop_n=None):
    f32 = mybir.dt.float32
    f32r = mybir.dt.float32r
    nc = bass.Bass()
    qr_d = nc.dram_tensor("qr", [5, NT * RS], f32r, kind="ExternalInput")
    o_d = nc.dram_tensor("o", [QT, NT * 8], f32, kind="ExternalOutput")

    with tile.TileContext(nc) as tc:
        with (
            tc.tile_pool(name="reg", bufs=4) as rpool,
            tc.tile_pool(name="acc", bufs=1) as apool,
            tc.tile_pool(name="psum", bufs=2, space="PSUM") as ppool,
            tc.tile_pool(name="scratch", bufs=1, space="PSUM") as spool,
        ):
            def body():
                vals = apool.tile([QT, NT * 8], f32, tag="vals")
                scratch = spool.tile([QT, 8], f32, tag="scratch")
                for t in range(NT):
                    rg = rpool.tile([5, RS], f32r, tag="rg")
                    # HWDGE region load; a [5, RS] transfer lands on a
                    # single HW queue/semaphore. Alternate the issuing
                    # engine (sync / scalar) — each engine's sequencer
                    # serializes its own DMAs, two engines overlap.
                    dma_eng = nc.sync if t % 2 == 0 else nc.scalar
                    dma_eng.dma_start(rg[:], qr_d[:, t * RS:(t + 1) * RS])
                    # 1x1 dummy matmul: absorbs the DMA-semaphore wait on
                    # PE so the real matmuls below carry at most one wait
                    # (the PSUM-slot recycle wait) — walrus limit.
                    nc.tensor.matmul(
                        scratch[0:1, 0:1],
                        rg[0:1, 0:1].bitcast(f32),
                        rg[0:1, 0:1].bitcast(f32),
                    )
                    # float32r runs the PE at 1 cycle/row (fp32 pays 4x);
                    # the ~1e-4 abs distance error is far below GUARD_EPS
                    # and irrelevant to the final mean.
                    w_ap = rg[:, 0:QT]
                    ps = ppool.tile([QT, W], f32, tag="ps")
                    for off in range(0, W, CHUNK):
                        sz = min(CHUNK, W - off)
                        nc.tensor.matmul(
                            ps[:, off:off + sz],
                            w_ap,
                            rg[:, QT + off:QT + off + sz],
                        )
                    # top-8 of -d2 (descending) = 8 smallest d2 of the
                    # whole window, straight into the output tile
                    nc.vector.max(vals[:, t * 8:(t + 1) * 8], ps[:])
                nc.sync.dma_start(o_d[:], vals[:])

            for _rep in range(loop_n or 1):  # loop_n: timing harness only
                body()

    # Walrus allows only ONE sync wait on a (self-loading) fp32 Matmult and
    # few on a Drain; Tile's wait pruning is disabled upstream, so prune:
    #  - Matmult: drop same-engine PE waits (PE executes matmuls in order).
    #  - Tail SP Drain: keep only the output-DMA (DMAHW) wait; the rest are
    #    transitively implied by the DMA's own waits.
    # sem updated by the final (output) DMA — the only wait the tail drain
    # needs: output-DMA-complete transitively implies DVE done, PE done,
    # and (via the dummy matmuls) every region DMA complete.
    last_dma_sem = None
    for blk in nc.m.functions[0].blocks:
        for inst in blk.instructions:
            if inst.opcode == "DMACopy" and inst.sync_info is not None:
                for u in inst.sync_info.on_update:
                    last_dma_sem = u.ant_name
    for blk in nc.m.functions[0].blocks:
        for inst in blk.instructions:
            si = inst.sync_info
            if si is None or len(si.on_wait) <= 1:
                continue
            if inst.opcode == "Matmult":
                kept = [w for w in si.on_wait if not w.ant_name.startswith("PE")]
                assert len(kept) <= 1, (
                    f"{inst.name}: {len(kept)} non-PE waits remain"
                )
                si.on_wait = kept
            elif inst.opcode == "DMACopy":
                # region-slot WAW: the PE wait (slot readers done, incl. the
                # dummy matmul that waited on the slot's previous DMA)
                # transitively implies the previous-DMA wait.
                if any(w.ant_name.startswith("PE") for w in si.on_wait):
                    kept = [
                        w for w in si.on_wait
                        if not w.ant_name.startswith(("DMASW", "DMAHW"))
                    ]
                    assert len(kept) <= 1, (
                        f"{inst.name}: {len(kept)} waits remain"
                    )
                    si.on_wait = kept
            elif inst.opcode == "Drain":
                kept = [w for w in si.on_wait if w.ant_name == last_dma_sem]
                if kept and len(kept) < len(si.on_wait):
                    si.on_wait = kept
    return nc


def _aug_q(qs):
    """[n, 3] queries -> [5, n] augmented lhsT columns."""
    out = np.empty((5, qs.shape[0]), dtype=np.float32)
    out[0:3] = 2.0 * qs.T
    out[3] = -np.sum(qs * qs, axis=-1)
    out[4] = -1.0
    return out


def _aug_r(rs):
    """[m, 3] refs -> [5, m] augmented rhs columns."""
    out = np.empty((5, rs.shape[0]), dtype=np.float32)
    out[0:3] = rs.T
    out[3] = 1.0
    out[4] = np.sum(rs * rs, axis=-1)
    return out


def _pack_inputs(query, ref):
    """Build per-core inputs + metadata for the guard/patch step.

    Returns (in_maps, meta) where meta[core] is a list of per-tile dicts:
    {qt: [128,3] query coords, b: batch, box: (xlo, xhi, ylo, yhi)}.
    """
    query = np.ascontiguousarray(np.asarray(query, dtype=np.float32))
    ref = np.ascontiguousarray(np.asarray(ref, dtype=np.float32))
    SPQ = N // NSTRIP  # queries per strip
    TPS = SPQ // QT    # tiles per strip
    in_maps = [
        {"qr": np.empty((5, NT * RS), dtype=np.float32)} for _ in range(N_CORES)
    ]
    meta = [[None] * NT for _ in range(N_CORES)]
    for b in range(B):
        q = query[b]
        r = ref[b]
        qo = np.argsort(q[:, 0], kind="stable")
        qs0 = q[qo]
        tile_idx = 0  # 0..63 within batch
        for st in range(NSTRIP):
            qstrip = qs0[st * SPQ:(st + 1) * SPQ]
            yo = np.argsort(qstrip[:, 1], kind="stable")
            qstrip = qstrip[yo]
            for tt in range(TPS):
                qt = qstrip[tt * QT:(tt + 1) * QT]
                xlo, xhi = float(qt[:, 0].min()), float(qt[:, 0].max())
                ylo, yhi = float(qt[:, 1].min()), float(qt[:, 1].max())
                # box-expansion radius needed to include each ref
                mx = np.maximum(np.maximum(xlo - r[:, 0], r[:, 0] - xhi), 0.0)
                my = np.maximum(np.maximum(ylo - r[:, 1], r[:, 1] - yhi), 0.0)
                mreq = np.maximum(mx, my)
                take = np.argpartition(mreq, W - 1)[:W]
                m_eff = float(mreq[take].max())
                # guard box must be fully covered by the taken refs; ties
                # at m_eff may be split by argpartition, so shrink a hair
                m_guard = max(m_eff * (1.0 - 1e-6) - 1e-9, 0.0)
                rslab = r[take]
                core = 2 * b + (0 if tile_idx < NT else 1)
                t = tile_idx % NT
                reg = in_maps[core]["qr"][:, t * RS:(t + 1) * RS]
                reg[:, 0:QT] = _aug_q(qt)
                reg[:, QT:QT + W] = _aug_r(rslab)
                meta[core][t] = {
                    "qt": qt,
                    "b": b,
                    "box": (xlo - m_guard, xhi + m_guard,
                            ylo - m_guard, yhi + m_guard),
                }
                tile_idx += 1
    return in_maps, meta


def _finish(results, meta, query, ref, K):
    """Merge device top-8 halves, apply exactness guard, patch failures."""
    ref = np.asarray(ref, dtype=np.float32)
    total = 0.0
    count = 0
    n_patched = 0
    for core in range(N_CORES):
        o = results[core]["o"].astype(np.float64)  # [128, NT*16], -d2 desc
        for t in range(NT):
            md = meta[core][t]
            cand = -o[:, t * 8:(t + 1) * 8]  # [128, 8] d2, ascending
            cand.sort(axis=1)
            top4 = cand[:, :4]
            v4 = top4[:, 3]
            qt = md["qt"].astype(np.float64)
            xlo, xhi, ylo, yhi = md["box"]
            gap = np.minimum.reduce([
                qt[:, 0] - xlo, xhi - qt[:, 0],
                qt[:, 1] - ylo, yhi - qt[:, 1],
            ])
            ok = gap * gap >= v4 + GUARD_EPS
            bad = np.where(~ok)[0]
            if len(bad):
                r = ref[md["b"]].astype(np.float64)
                for p in bad:
                    qrow = qt[p]
                    d2 = np.sum((r - qrow) ** 2, axis=1)
                    top4[p] = np.sort(np.partition(d2, 3)[:4])
                n_patched += len(bad)
            total += float(top4.sum())
            count += QT * 4
    assert count == B * N * int(K)
    _finish.n_patched = n_patched
    return total / count


def kernel(query, ref, K):
    assert int(K) == 4, f"kernel hardcodes K=4, got {K}"
    qa = np.asarray(query)
    assert qa.shape == (B, N, D)
    in_maps, meta = _pack_inputs(query, ref)
    nc = _build_nc()
    res = run_bass_kernel_spmd(nc, in_maps, core_ids=list(range(N_CORES)))
    kernel._last = res  # for test harness introspection
    mean = _finish(res.results, meta, query, ref, K)
    return np.float32(mean)


# revision 30
# speedup vs baseline: 2.1692x; 1.0372x over previous
"""Chamfer/KNN top-4 mean distance kernel for Trainium2 (8 NeuronCores).

Problem: query [4, 8192, 3], ref [4, 8192, 3], K=4.
  d2[b,n,m] = ||q_bn - r_bm||^2 ; answer = mean over (b,n) of the 4 smallest
  d2[b,n,:] values.

Strategy:
  - Augmented-matmul distances: q' = [2q, -||q||^2, -1], r' = [r, 1, ||r||^2]
    so the PE matmul q'^T @ r' yields NEGATED squared distances in PSUM, and
    the DVE `max` (hardware top-8, descending) extracts the 4 smallest d2
    per query in one pass.
  - 2D locality sharding (host-side layout): queries of each batch are
    sorted into 8 x-strips, y-sorted within each strip, and cut into
    128-query tiles. Each tile is paired with the W=2048 refs closest to
    its bounding box (smallest box-expansion radius). A per-query guard —
    dist(q, box boundary)^2 >= found 4th-smallest d2 — proves exactness;
    the ~0.3% of queries failing the guard are recomputed exactly on the
    host against the full ref set.
  - 8 cores: 2 per batch, 32 tiles each. Per tile: one region DMA
    ([5, 128+W]), a 1x1 dummy matmul that absorbs the DMA semaphore wait
    (walrus allows a single sync wait on fp32 Matmult), 4 matmuls of
    [128, 512] into two 2-bank PSUM buffers, and two `max` ops writing
    top-8 candidates straight into the output tile.
  - Host merges each tile's two top-8 halves, applies the guard, patches
    failures, and averages.
"""

import numpy as np

import concourse.bass as bass
import concourse.mybir as mybir
import concourse.tile as tile
from concourse.bass_utils import run_bass_kernel_spmd

N_CORES = 8
B, N, M, D = 4, 8192, 8192, 3
NQ = 4096       # query rows per core
QT = 128        # queries per tile (PSUM partition dim)
NT = NQ // QT   # 32 tiles per core
W = 1280        # refs per tile window
CHUNK = 512     # matmul free dim (one PSUM bank, fp32)
RS = QT + W     # region stride: [queries | window refs]
GUARD_EPS = 1e-3


def _build_nc(loop_n=None):
    f32 = mybir.dt.float32
    f32r = mybir.dt.float32r
    nc = bass.Bass()
    qr_d = nc.dram_tensor("qr", [5, NT * RS], f32r, kind="ExternalInput")
    o_d = nc.dram_tensor("o", [QT, NT * 8], f32, kind="ExternalOutput")

    with tile.TileContext(nc) as tc:
        with (
            tc.tile_pool(name="reg", bufs=4) as rpool,
            tc.tile_pool(name="acc", bufs=1) as apool,
            tc.tile_pool(name="psum", bufs=2, space="PSUM") as ppool,
            tc.tile_pool(name="scratch", bufs=1, space="PSUM") as spool,
        ):
            def body():
                vals = apool.tile([QT, NT * 8], f32, tag="vals")
                scratch = spool.tile([QT, 8], f32, tag="scratch")
                for t in range(NT):
                    rg = rpool.tile([5, RS], f32r, tag="rg")
                    # HWDGE region load; a [5, RS] transfer lands on a
                    # single HW queue/semaphore. Alternate the issuing
                    # engine (sync / scalar) — each engine's sequencer
                    # serializes its own DMAs, two engines overlap.
                    # Tile 0 is on the critical path: split it across both
                    # engines (two dummy matmuls absorb the two sems).
                    halves = [(0, RS)] if t > 0 else [(0, RS // 2), (RS // 2, RS)]
                    for i, (a, z) in enumerate(halves):
                        eng = nc.sync if (t + i) % 2 == 0 else nc.scalar
                        eng.dma_start(
                            rg[:, a:z], qr_d[:, t * RS + a:t * RS + z]
                        )
                        # 1x1 dummy matmul: absorbs the DMA-semaphore wait
                        # on PE so the real matmuls below carry at most one
                        # wait (the PSUM-slot recycle wait) — walrus limit.
                        nc.tensor.matmul(
                            scratch[0:1, i:i + 1],
                            rg[0:1, a:a + 1].bitcast(f32),
                            rg[0:1, a:a + 1].bitcast(f32),
                        )
                    # float32r runs the PE at 1 cycle/row (fp32 pays 4x);
                    # the ~1e-4 abs distance error is far below GUARD_EPS
                    # and irrelevant to the final mean.
                    w_ap = rg[:, 0:QT]
                    ps = ppool.tile([QT, W], f32, tag="ps")
                    for off in range(0, W, CHUNK):
                        sz = min(CHUNK, W - off)
                        nc.tensor.matmul(
                            ps[:, off:off + sz],
                            w_ap,
                            rg[:, QT + off:QT + off + sz],
                        )
                    # top-8 of -d2 (descending) = 8 smallest d2 of the
                    # whole window, straight into the output tile
                    nc.vector.max(vals[:, t * 8:(t + 1) * 8], ps[:])
                nc.sync.dma_start(o_d[:], vals[:])

            for _rep in range(loop_n or 1):  # loop_n: timing harness only
                body()

    # Walrus allows only ONE sync wait on a (self-loading) fp32 Matmult and
    # few on a Drain; Tile's wait pruning is disabled upstream, so prune:
    #  - Matmult: drop same-engine PE waits (PE executes matmuls in order).
    #  - Tail SP Drain: keep only the output-DMA (DMAHW) wait; the rest are
    #    transitively implied by the DMA's own waits.
    # sem updated by the final (output) DMA — the only wait the tail drain
    # needs: output-DMA-complete transitively implies DVE done, PE done,
    # and (via the dummy matmuls) every region DMA complete.
    last_dma_sem = None
    for blk in nc.m.functions[0].blocks:
        for inst in blk.instructions:
            if inst.opcode == "DMACopy" and inst.sync_info is not None:
                for u in inst.sync_info.on_update:
                    last_dma_sem = u.ant_name
    for blk in nc.m.functions[0].blocks:
        for inst in blk.instructions:
            si = inst.sync_info
            if si is None or len(si.on_wait) <= 1:
                continue
            if inst.opcode == "Matmult":
                kept = [w for w in si.on_wait if not w.ant_name.startswith("PE")]
                assert len(kept) <= 1, (
                    f"{inst.name}: {len(kept)} non-PE waits remain"
                )
                si.on_wait = kept
            elif inst.opcode == "DMACopy":
                # region-slot WAW: the PE wait (slot readers done, incl. the
                # dummy matmul that waited on the slot's previous DMA)
                # transitively implies the previous-DMA wait.
                if any(w.ant_name.startswith("PE") for w in si.on_wait):
                    kept = [
                        w for w in si.on_wait
                        if not w.ant_name.startswith(("DMASW", "DMAHW"))
                    ]
                    assert len(kept) <= 1, (
                        f"{inst.name}: {len(kept)} waits remain"
                    )
                    si.on_wait = kept
            elif inst.opcode == "Drain":
                kept = [w for w in si.on_wait if w.ant_name == last_dma_sem]
                if kept and len(kept) < len(si.on_wait):
                    si.on_wait = kept
    return nc


def _aug_q(qs):
    """[n, 3] queries -> [5, n] augmented lhsT columns."""
    out = np.empty((5, qs.shape[0]), dtype=np.float32)
    out[0:3] = 2.0 * qs.T
    out[3] = -np.sum(qs * qs, axis=-1)
    out[4] = -1.0
    return out


def _aug_r(rs):
    """[m, 3] refs -> [5, m] augmented rhs columns."""
    out = np.empty((5, rs.shape[0]), dtype=np.float32)
    out[0:3] = rs.T
    out[3] = 1.0
    out[4] = np.sum(rs * rs, axis=-1)
    return out


def _pack_inputs(query, ref):
    """Build per-core inputs + metadata for the guard/patch step.

    Returns (in_maps, meta) where meta[core] is a list of per-tile dicts:
    {qt: [128,3] query coords, b: batch, box: (xlo, xhi, ylo, yhi)}.
    """
    query = np.ascontiguousarray(np.asarray(query, dtype=np.float32))
    ref = np.ascontiguousarray(np.asarray(ref, dtype=np.float32))
    in_maps = [
        {"qr": np.empty((5, NT * RS), dtype=np.float32)} for _ in range(N_CORES)
    ]
    meta = [[None] * NT for _ in range(N_CORES)]
    for b in range(B):
        q = query[b]
        r = ref[b]
        qs = q[np.argsort(q[:, 0], kind="stable")]
        tile_idx = 0  # 0..63 within batch
        for sx in range(4):
            qx = qs[sx * (N // 4):(sx + 1) * (N // 4)]
            qx = qx[np.argsort(qx[:, 1], kind="stable")]
            for sy in range(4):
                qy = qx[sy * (N // 16):(sy + 1) * (N // 16)]
                qy = qy[np.argsort(qy[:, 2], kind="stable")]
                for sz in range(4):
                    qt = qy[sz * QT:(sz + 1) * QT]
                    lo = qt.min(0)
                    hi = qt.max(0)
                    # L-inf box-expansion radius needed to include each ref
                    exc = np.maximum(
                        np.maximum(lo[None, :] - r, r - hi[None, :]), 0.0
                    )
                    mreq = exc.max(1)
                    take = np.argpartition(mreq, W - 1)[:W]
                    m_eff = float(mreq[take].max())
                    # guard box must be fully covered by the taken refs;
                    # ties at m_eff may be split, so shrink a hair
                    m_guard = max(m_eff * (1.0 - 1e-6) - 1e-9, 0.0)
                    rslab = r[take]
                    core = 2 * b + (0 if tile_idx < NT else 1)
                    t = tile_idx % NT
                    reg = in_maps[core]["qr"][:, t * RS:(t + 1) * RS]
                    reg[:, 0:QT] = _aug_q(qt)
                    reg[:, QT:QT + W] = _aug_r(rslab)
                    meta[core][t] = {
                        "qt": qt,
                        "b": b,
                        "lo": lo - m_guard,
                        "hi": hi + m_guard,
                    }
                    tile_idx += 1
    return in_maps, meta


def _finish(results, meta, query, ref, K):
    """Merge device top-8 halves, apply exactness guard, patch failures."""
    ref = np.asarray(ref, dtype=np.float32)
    total = 0.0
    count = 0
    n_patched = 0
    for core in range(N_CORES):
        o = results[core]["o"].astype(np.float64)  # [128, NT*16], -d2 desc
        for t in range(NT):
            md = meta[core][t]
            cand = -o[:, t * 8:(t + 1) * 8]  # [128, 8] d2, ascending
            cand.sort(axis=1)
            top4 = cand[:, :4]
            v4 = top4[:, 3]
            qt = md["qt"].astype(np.float64)
            lo = md["lo"].astype(np.float64)
            hi = md["hi"].astype(np.float64)
            gap = np.minimum((qt - lo[None, :]).min(1),
                             (hi[None, :] - qt).min(1))
            ok = gap * gap >= v4 + GUARD_EPS
            bad = np.where(~ok)[0]
            if len(bad):
                r = ref[md["b"]].astype(np.float64)
                for p in bad:
                    qrow = qt[p]
                    d2 = np.sum((r - qrow) ** 2, axis=1)
                    top4[p] = np.sort(np.partition(d2, 3)[:4])
                n_patched += len(bad)
            total += float(top4.sum())
            count += QT * 4
    assert count == B * N * int(K)
    _finish.n_patched = n_patched
    return total / count


def kernel(query, ref, K):
    assert int(K) == 4, f"kernel hardcodes K=4, got {K}"
    qa = np.asarray(query)
    assert qa.shape == (B, N, D)
    in_maps, meta = _pack_inputs(query, ref)
    nc = _build_nc()
    res = run_bass_kernel_spmd(nc, in_maps, core_ids=list(range(N_CORES)))
    kernel._last = res  # for test harness introspection
    mean = _finish(res.results, meta, query, ref, K)
    return np.float32(mean)


# revision 31
# speedup vs baseline: 2.3471x; 1.0820x over previous
"""Chamfer/KNN top-4 mean distance kernel for Trainium2 (8 NeuronCores).

Problem: query [4, 8192, 3], ref [4, 8192, 3], K=4.
  d2[b,n,m] = ||q_bn - r_bm||^2 ; answer = mean over (b,n) of the 4 smallest
  d2[b,n,:] values.

Strategy:
  - Augmented-matmul distances: q' = [2q, -||q||^2, -1], r' = [r, 1, ||r||^2]
    so the PE matmul q'^T @ r' yields NEGATED squared distances in PSUM, and
    the DVE `max` (hardware top-8, descending) extracts the 4 smallest d2
    per query in one pass.
  - 2D locality sharding (host-side layout): queries of each batch are
    sorted into 8 x-strips, y-sorted within each strip, and cut into
    128-query tiles. Each tile is paired with the W=2048 refs closest to
    its bounding box (smallest box-expansion radius). A per-query guard —
    dist(q, box boundary)^2 >= found 4th-smallest d2 — proves exactness;
    the ~0.3% of queries failing the guard are recomputed exactly on the
    host against the full ref set.
  - 8 cores: 2 per batch, 32 tiles each. Per tile: one region DMA
    ([5, 128+W]), a 1x1 dummy matmul that absorbs the DMA semaphore wait
    (walrus allows a single sync wait on fp32 Matmult), 4 matmuls of
    [128, 512] into two 2-bank PSUM buffers, and two `max` ops writing
    top-8 candidates straight into the output tile.
  - Host merges each tile's two top-8 halves, applies the guard, patches
    failures, and averages.
"""

import numpy as np

import concourse.bass as bass
import concourse.mybir as mybir
import concourse.tile as tile
from concourse.bass_utils import run_bass_kernel_spmd

N_CORES = 8
B, N, M, D = 4, 8192, 8192, 3
NQ = 4096       # query rows per core
QT = 128        # queries per tile (PSUM partition dim)
NT = NQ // QT   # 32 tiles per core
W = 1152        # refs per tile window
CHUNK = 512     # matmul free dim (one PSUM bank, fp32)
RS = QT + W     # region stride: [queries | window refs]
GUARD_EPS = 1e-3


def _build_nc(loop_n=None):
    f32 = mybir.dt.float32
    f32r = mybir.dt.float32r
    nc = bass.Bass()
    qr_d = nc.dram_tensor("qr", [5, NT * RS], f32r, kind="ExternalInput")
    o_d = nc.dram_tensor("o", [QT, NT * 8], f32, kind="ExternalOutput")

    with tile.TileContext(nc) as tc:
        with (
            tc.tile_pool(name="reg", bufs=4) as rpool,
            tc.tile_pool(name="acc", bufs=1) as apool,
            tc.tile_pool(name="psum", bufs=2, space="PSUM") as ppool,
            tc.tile_pool(name="scratch", bufs=1, space="PSUM") as spool,
        ):
            def body():
                vals = apool.tile([QT, NT * 8], f32, tag="vals")
                scratch = spool.tile([QT, 8], f32, tag="scratch")
                for t in range(NT):
                    rg = rpool.tile([5, RS], f32r, tag="rg")
                    # HWDGE region load; a [5, RS] transfer lands on a
                    # single HW queue/semaphore. Alternate the issuing
                    # engine (sync / scalar) — each engine's sequencer
                    # serializes its own DMAs, two engines overlap.
                    # Tile 0 is on the critical path: split it across both
                    # engines (two dummy matmuls absorb the two sems).
                    halves = [(0, RS)] if t > 0 else [(0, RS // 2), (RS // 2, RS)]
                    for i, (a, z) in enumerate(halves):
                        eng = nc.sync if (t + i) % 2 == 0 else nc.scalar
                        eng.dma_start(
                            rg[:, a:z], qr_d[:, t * RS + a:t * RS + z]
                        )
                        # 1x1 dummy matmul: absorbs the DMA-semaphore wait
                        # on PE so the real matmuls below carry at most one
                        # wait (the PSUM-slot recycle wait) — walrus limit.
                        nc.tensor.matmul(
                            scratch[0:1, i:i + 1],
                            rg[0:1, a:a + 1].bitcast(f32),
                            rg[0:1, a:a + 1].bitcast(f32),
                        )
                    # float32r runs the PE at 1 cycle/row (fp32 pays 4x);
                    # the ~1e-4 abs distance error is far below GUARD_EPS
                    # and irrelevant to the final mean.
                    w_ap = rg[:, 0:QT]
                    ps = ppool.tile([QT, W], f32, tag="ps")
                    for off in range(0, W, CHUNK):
                        sz = min(CHUNK, W - off)
                        nc.tensor.matmul(
                            ps[:, off:off + sz],
                            w_ap,
                            rg[:, QT + off:QT + off + sz],
                        )
                    # top-8 of -d2 (descending) = 8 smallest d2 of the
                    # whole window, straight into the output tile
                    nc.vector.max(vals[:, t * 8:(t + 1) * 8], ps[:])
                nc.sync.dma_start(o_d[:], vals[:])

            for _rep in range(loop_n or 1):  # loop_n: timing harness only
                body()

    # Walrus allows only ONE sync wait on a (self-loading) fp32 Matmult and
    # few on a Drain; Tile's wait pruning is disabled upstream, so prune:
    #  - Matmult: drop same-engine PE waits (PE executes matmuls in order).
    #  - Tail SP Drain: keep only the output-DMA (DMAHW) wait; the rest are
    #    transitively implied by the DMA's own waits.
    # sem updated by the final (output) DMA — the only wait the tail drain
    # needs: output-DMA-complete transitively implies DVE done, PE done,
    # and (via the dummy matmuls) every region DMA complete.
    last_dma_sem = None
    for blk in nc.m.functions[0].blocks:
        for inst in blk.instructions:
            if inst.opcode == "DMACopy" and inst.sync_info is not None:
                for u in inst.sync_info.on_update:
                    last_dma_sem = u.ant_name
    for blk in nc.m.functions[0].blocks:
        for inst in blk.instructions:
            si = inst.sync_info
            if si is None or len(si.on_wait) <= 1:
                continue
            if inst.opcode == "Matmult":
                kept = [w for w in si.on_wait if not w.ant_name.startswith("PE")]
                assert len(kept) <= 1, (
                    f"{inst.name}: {len(kept)} non-PE waits remain"
                )
                si.on_wait = kept
            elif inst.opcode == "DMACopy":
                # region-slot WAW: the PE wait (slot readers done, incl. the
                # dummy matmul that waited on the slot's previous DMA)
                # transitively implies the previous-DMA wait.
                if any(w.ant_name.startswith("PE") for w in si.on_wait):
                    kept = [
                        w for w in si.on_wait
                        if not w.ant_name.startswith(("DMASW", "DMAHW"))
                    ]
                    assert len(kept) <= 1, (
                        f"{inst.name}: {len(kept)} waits remain"
                    )
                    si.on_wait = kept
            elif inst.opcode == "Drain":
                kept = [w for w in si.on_wait if w.ant_name == last_dma_sem]
                if kept and len(kept) < len(si.on_wait):
                    si.on_wait = kept
    return nc


def _aug_q(qs):
    """[n, 3] queries -> [5, n] augmented lhsT columns."""
    out = np.empty((5, qs.shape[0]), dtype=np.float32)
    out[0:3] = 2.0 * qs.T
    out[3] = -np.sum(qs * qs, axis=-1)
    out[4] = -1.0
    return out


def _aug_r(rs):
    """[m, 3] refs -> [5, m] augmented rhs columns."""
    out = np.empty((5, rs.shape[0]), dtype=np.float32)
    out[0:3] = rs.T
    out[3] = 1.0
    out[4] = np.sum(rs * rs, axis=-1)
    return out


def _pack_inputs(query, ref):
    """Build per-core inputs + metadata for the guard/patch step.

    Returns (in_maps, meta) where meta[core] is a list of per-tile dicts:
    {qt: [128,3] query coords, b: batch, box: (xlo, xhi, ylo, yhi)}.
    """
    query = np.ascontiguousarray(np.asarray(query, dtype=np.float32))
    ref = np.ascontiguousarray(np.asarray(ref, dtype=np.float32))
    in_maps = [
        {"qr": np.empty((5, NT * RS), dtype=np.float32)} for _ in range(N_CORES)
    ]
    meta = [[None] * NT for _ in range(N_CORES)]
    for b in range(B):
        q = query[b]
        r = ref[b]
        qs = q[np.argsort(q[:, 0], kind="stable")]
        tile_idx = 0  # 0..63 within batch
        for sx in range(4):
            qx = qs[sx * (N // 4):(sx + 1) * (N // 4)]
            qx = qx[np.argsort(qx[:, 1], kind="stable")]
            for sy in range(4):
                qy = qx[sy * (N // 16):(sy + 1) * (N // 16)]
                qy = qy[np.argsort(qy[:, 2], kind="stable")]
                for sz in range(4):
                    qt = qy[sz * QT:(sz + 1) * QT]
                    lo = qt.min(0)
                    hi = qt.max(0)
                    # L-inf box-expansion radius needed to include each ref
                    exc = np.maximum(
                        np.maximum(lo[None, :] - r, r - hi[None, :]), 0.0
                    )
                    mreq = exc.max(1)
                    take = np.argpartition(mreq, W - 1)[:W]
                    m_eff = float(mreq[take].max())
                    # guard box must be fully covered by the taken refs;
                    # ties at m_eff may be split, so shrink a hair
                    m_guard = max(m_eff * (1.0 - 1e-6) - 1e-9, 0.0)
                    rslab = r[take]
                    core = 2 * b + (0 if tile_idx < NT else 1)
                    t = tile_idx % NT
                    reg = in_maps[core]["qr"][:, t * RS:(t + 1) * RS]
                    reg[:, 0:QT] = _aug_q(qt)
                    reg[:, QT:QT + W] = _aug_r(rslab)
                    meta[core][t] = {
                        "qt": qt,
                        "b": b,
                        "lo": lo - m_guard,
                        "hi": hi + m_guard,
                    }
                    tile_idx += 1
    return in_maps, meta


def _finish(results, meta, query, ref, K):
    """Merge device top-8 halves, apply exactness guard, patch failures."""
    ref = np.asarray(ref, dtype=np.float32)
    total = 0.0
    count = 0
    n_patched = 0
    for core in range(N_CORES):
        o = results[core]["o"].astype(np.float64)  # [128, NT*16], -d2 desc
        for t in range(NT):
            md = meta[core][t]
            cand = -o[:, t * 8:(t + 1) * 8]  # [128, 8] d2, ascending
            cand.sort(axis=1)
            top4 = cand[:, :4]
            v4 = top4[:, 3]
            qt = md["qt"].astype(np.float64)
            lo = md["lo"].astype(np.float64)
            hi = md["hi"].astype(np.float64)
            gap = np.minimum((qt - lo[None, :]).min(1),
                             (hi[None, :] - qt).min(1))
            ok = gap * gap >= v4 + GUARD_EPS
            bad = np.where(~ok)[0]
            if len(bad):
                r = ref[md["b"]].astype(np.float64)
                for p in bad:
                    qrow = qt[p]
                    d2 = np.sum((r - qrow) ** 2, axis=1)
                    top4[p] = np.sort(np.partition(d2, 3)[:4])
                n_patched += len(bad)
            total += float(top4.sum())
            count += QT * 4
    assert count == B * N * int(K)
    _finish.n_patched = n_patched
    return total / count


def kernel(query, ref, K):
    assert int(K) == 4, f"kernel hardcodes K=4, got {K}"
    qa = np.asarray(query)
    assert qa.shape == (B, N, D)
    in_maps, meta = _pack_inputs(query, ref)
    nc = _build_nc()
    res = run_bass_kernel_spmd(nc, in_maps, core_ids=list(range(N_CORES)))
    kernel._last = res  # for test harness introspection
    mean = _finish(res.results, meta, query, ref, K)
    return np.float32(mean)
